# revision 13
# baseline (speedup 1.0000x reference)
"""AttentionRNN (BiGRU + tanh attention pooling) on 8 Trainium2 NeuronCores.

Sharding: data-parallel over batch (32 samples -> 4 per core). Two NEFF
launches:
  A) input-gate projections (bf16 matmuls, fp32 accum) + forward and
     backward GRU scans (backward runs on host-pre-reversed input, so both
     scans are plain in-order recurrences; no masking on device - padded
     tail steps compute garbage that is discarded later).
  B) attention: proj = tanh(Wp @ y + bp), scores = u . proj (+ host-built
     -1e30 mask), softmax over time, weighted = sum_t attn_t * y_t.
Between launches the host un-reverses the backward outputs per sample
(ragged lengths) and zero-pads; after B the host assembles full outputs.

Recurrent matmul layout: out[gate, sample] = Whh^T.T @ h, gate tiles
stationary (bf16 -> fast weight load), batch (4) on the moving free dim.
"""

import os
import sys
import types

import numpy as np
import ml_dtypes

BF16 = ml_dtypes.bfloat16

B, T, I, H, A = 32, 1024, 512, 512, 256
NCORES = 8
BPC = B // NCORES          # samples per core (4)
KT = I // 128              # 4 k-tiles for the 512-dim contractions
MT = 3 * H // 128          # 12 gate m-tiles
CHUNK = 64                 # scan steps per xg/y SBUF chunk


def _install_ntff_shim():
    try:
        from antenv import axon_hooks  # noqa: F401
        return
    except ImportError:
        pass
    try:
        import antenv
        from trn_agent_boot.trn_boot import _ntff_profile_via_ctypes
        hook = _ntff_profile_via_ctypes('/opt/axon/libaxon_pjrt.so')
        m = types.ModuleType('antenv.axon_hooks')
        m.get_axon_ntff_profile_hook = lambda: hook
        m.set_axon_ntff_profile_hook = lambda h: None
        sys.modules['antenv.axon_hooks'] = m
        antenv.axon_hooks = m
    except Exception:
        pass


_install_ntff_shim()

import concourse.bass as bass  # noqa: E402
import concourse.mybir as mybir  # noqa: E402
import concourse.tile as tile  # noqa: E402
from concourse import bacc  # noqa: E402
from concourse.bass_utils import run_bass_kernel_spmd as _run_spmd  # noqa: E402


def run_bass_kernel_spmd(nc, in_maps, core_ids, trace=False):
    import time as _time
    last = None
    for attempt in range(3):
        try:
            return _run_spmd(nc, in_maps, core_ids=core_ids, trace=trace)
        except Exception as e:  # wedged-device flakes on shared cores
            last = e
            _time.sleep(3.0)
    raise last

F32 = mybir.dt.float32
BF = mybir.dt.bfloat16
AF = mybir.ActivationFunctionType

_cache = {}
LAST_EXEC_NS = None


# ---------------------------------------------------------------- launch A
def _build_a():
    nc = bacc.Bacc("TRN2", target_bir_lowering=False, debug=False,
                   num_devices=NCORES)
    xT = nc.dram_tensor("xT", [128, KT, BPC * T], BF, kind="ExternalInput")
    xrT = nc.dram_tensor("xrT", [128, KT, BPC * T], BF, kind="ExternalInput")
    wih = [nc.dram_tensor(f"wih{d}", [128, KT * 3 * H], BF, kind="ExternalInput")
           for d in range(2)]
    whh = [nc.dram_tensor(f"whh{d}", [128, KT * 3 * H], BF, kind="ExternalInput")
           for d in range(2)]
    bprj = [nc.dram_tensor(f"bprj{d}", [128, MT], F32, kind="ExternalInput")
            for d in range(2)]
    bhn = [nc.dram_tensor(f"bhn{d}", [128, KT, BPC], F32, kind="ExternalInput")
           for d in range(2)]
    y32 = nc.dram_tensor("y32", [128, 2, KT, BPC, T], F32, kind="ExternalOutput")
    ybf = nc.dram_tensor("ybf", [128, 2, KT, BPC, T], BF, kind="ExternalOutput")
    xg = nc.dram_tensor("xg", [2, BPC, MT, 128, T], F32, kind="Internal")

    with tile.TileContext(nc) as tc:
        # ---- phase 1: input projections xg = Wih @ x^T + (bih [+ bhh]) ----
        with tc.tile_pool(name="pw", bufs=1) as pw, \
             tc.tile_pool(name="px", bufs=1) as px, \
             tc.tile_pool(name="pb", bufs=1) as pb, \
             tc.tile_pool(name="pp", bufs=4, space="PSUM") as pp, \
             tc.tile_pool(name="po", bufs=4) as po:
            x_sb = []
            w_sb = []
            b_sb = []
            for d in range(2):
                xs = px.tile([128, KT, BPC * T], BF, tag=f"x{d}")
                nc.sync.dma_start(xs[:], (xT if d == 0 else xrT).ap())
                x_sb.append(xs)
                ws = pw.tile([128, KT * 3 * H], BF, tag=f"w{d}")
                nc.sync.dma_start(ws[:], wih[d].ap())
                w_sb.append(ws)
                bs = pb.tile([128, MT], F32, tag=f"b{d}")
                nc.sync.dma_start(bs[:], bprj[d].ap())
                b_sb.append(bs)
            PC = min(512, T)
            nch = BPC * T // PC
            for d in range(2):
                for m in range(MT):
                    for ch in range(nch):
                        ps = pp.tile([128, PC], F32, tag="ps")
                        for k in range(KT):
                            nc.tensor.matmul(
                                ps[:],
                                w_sb[d][:, k * 3 * H + m * 128:
                                        k * 3 * H + (m + 1) * 128],
                                x_sb[d][:, k, ch * PC:(ch + 1) * PC],
                                start=(k == 0), stop=(k == KT - 1))
                        ot = po.tile([128, PC], F32, tag="ot")
                        nc.vector.tensor_scalar_add(ot[:], ps[:],
                                                    b_sb[d][:, m:m + 1])
                        s = (ch * PC) // T
                        tl = (ch * PC) % T
                        nc.sync.dma_start(xg.ap()[d, s, m, :, tl:tl + PC], ot[:])

        # ---- phase 2: the two GRU scans ----
        tc.strict_bb_all_engine_barrier()
        with tc.tile_pool(name="sw", bufs=1) as sw, \
             tc.tile_pool(name="sb0", bufs=1) as sb0, \
             tc.tile_pool(name="sxg", bufs=2) as sxg, \
             tc.tile_pool(name="sy32", bufs=2) as sy32, \
             tc.tile_pool(name="sybf", bufs=2) as sybf, \
             tc.tile_pool(name="sps", bufs=2, space="PSUM") as sps, \
             tc.tile_pool(name="stmp", bufs=3) as stmp:
            whh_sb = []
            bhn_sb = []
            for d in range(2):
                ws = sw.tile([128, KT * 3 * H], BF, tag=f"whh{d}")
                nc.sync.dma_start(ws[:], whh[d].ap())
                whh_sb.append(ws)
                bs = sw.tile([128, KT, BPC], F32, tag=f"bhn{d}")
                nc.sync.dma_start(bs[:], bhn[d].ap())
                bhn_sb.append(bs)
            z32 = sb0.tile([128, KT, BPC], F32, tag="z32")
            nc.vector.memset(z32[:], 0.0)
            zbf = sb0.tile([128, KT, BPC], BF, tag="zbf")
            nc.vector.memset(zbf[:], 0.0)

            prev32 = [z32, z32]
            prevbf = [zbf, zbf]
            xgc = [None, None]
            yb32 = [None, None]
            ybbf = [None, None]
            nchunks = T // CHUNK
            for ci in range(nchunks):
                t0 = ci * CHUNK
                for d in range(2):
                    xgc[d] = sxg.tile([128, MT, BPC, CHUNK], F32, tag=f"xg{d}", name=f"xgc{d}_{ci}")
                    for s in range(BPC):
                        for m in range(MT):
                            nc.sync.dma_start(
                                xgc[d][:, m, s, :],
                                xg.ap()[d, s, m, :, t0:t0 + CHUNK])
                    yb32[d] = sy32.tile([128, KT, BPC, CHUNK], F32, tag=f"y32{d}", name=f"yb32_{d}_{ci}")
                    ybbf[d] = sybf.tile([128, KT, BPC, CHUNK], BF, tag=f"ybf{d}", name=f"ybbf_{d}_{ci}")
                for tl in range(CHUNK):
                    for d in range(2):
                        first = (ci == 0 and tl == 0)
                        if tl == 0:
                            hbf_t, h32_t = prevbf[d], prev32[d]
                        else:
                            hbf_t, h32_t = ybbf[d], yb32[d]

                        def hbf_rhs(k):
                            if first:
                                return hbf_t[:, k, :]
                            if tl == 0:
                                return hbf_t[:, k, :, CHUNK - 1]
                            return hbf_t[:, k, :, tl - 1]

                        if first:
                            h32_ap = h32_t[:, :, :]
                        elif tl == 0:
                            h32_ap = h32_t[:, :, :, CHUNK - 1]
                        else:
                            h32_ap = h32_t[:, :, :, tl - 1]
                        ps = sps.tile([128, MT, BPC], F32, tag=f"ps{d}",
                                      name=f"ps{d}_{ci}_{tl}")
                        for m in range(MT):
                            for k in range(KT):
                                nc.tensor.matmul(
                                    ps[:, m, :],
                                    whh_sb[d][:, k * 3 * H + m * 128:
                                              k * 3 * H + (m + 1) * 128],
                                    hbf_rhs(k),
                                    start=(k == 0), stop=(k == KT - 1))
                        xga = xgc[d][:, :, :, tl]
                        # n-gate hidden bias (inside the r* product)
                        nc.vector.tensor_add(ps[:, 8:12, :], ps[:, 8:12, :],
                                             bhn_sb[d][:])
                        srz = stmp.tile([128, 8, BPC], F32, tag=f"srz{d}")
                        nc.vector.tensor_add(srz[:], ps[:, 0:8, :],
                                             xga[:, 0:8, :])
                        rz = stmp.tile([128, 8, BPC], F32, tag=f"rz{d}")
                        nc.scalar.activation(rz[:], srz[:], AF.Sigmoid)
                        tn = stmp.tile([128, KT, BPC], F32, tag=f"tn{d}")
                        nc.vector.tensor_mul(tn[:], rz[:, 0:4, :],
                                             ps[:, 8:12, :])
                        nc.vector.tensor_add(tn[:], tn[:], xga[:, 8:12, :])
                        n_t = stmp.tile([128, KT, BPC], F32, tag=f"n{d}")
                        nc.scalar.activation(n_t[:], tn[:], AF.Tanh)
                        dd = stmp.tile([128, KT, BPC], F32, tag=f"dd{d}")
                        nc.vector.tensor_sub(dd[:], h32_ap, n_t[:])
                        nc.vector.tensor_mul(dd[:], rz[:, 4:8, :], dd[:])
                        nc.vector.tensor_add(yb32[d][:, :, :, tl], n_t[:], dd[:])
                        nc.vector.tensor_copy(ybbf[d][:, :, :, tl],
                                              yb32[d][:, :, :, tl])
                for d in range(2):
                    nc.sync.dma_start(y32.ap()[:, d, :, :, t0:t0 + CHUNK],
                                      yb32[d][:])
                    nc.sync.dma_start(ybf.ap()[:, d, :, :, t0:t0 + CHUNK],
                                      ybbf[d][:])
                    prev32[d] = yb32[d]
                    prevbf[d] = ybbf[d]
    nc.compile()
    return nc


# ---------------------------------------------------------------- launch B
def _build_b(stage=3):
    nc = bacc.Bacc("TRN2", target_bir_lowering=False, debug=False,
                   num_devices=NCORES)
    ybf = nc.dram_tensor("ybf", [128, 2, KT, BPC, T], BF, kind="ExternalInput")
    wp = nc.dram_tensor("wp", [128, 8 * A], BF, kind="ExternalInput")
    bp2 = nc.dram_tensor("bp2", [128, 2], F32, kind="ExternalInput")
    u2 = nc.dram_tensor("u2", [128, 2], BF, kind="ExternalInput")
    smask = nc.dram_tensor("smask", [BPC, T], F32, kind="ExternalInput")
    wout = nc.dram_tensor("wout", [BPC, 128, 8], F32, kind="ExternalOutput")
    aout = nc.dram_tensor("aout", [BPC, T], F32, kind="ExternalOutput")

    PC2 = min(512, T)
    NC2 = T // PC2
    with tile.TileContext(nc) as tc:
        with tc.tile_pool(name="py", bufs=1) as py, \
             tc.tile_pool(name="pw", bufs=1) as pw, \
             tc.tile_pool(name="ppp", bufs=2, space="PSUM") as ppp, \
             tc.tile_pool(name="pps", bufs=2, space="PSUM") as pps, \
             tc.tile_pool(name="ppb", bufs=2, space="PSUM") as ppb, \
             tc.tile_pool(name="pt", bufs=4) as pt, \
             tc.tile_pool(name="psc", bufs=1) as psc:
            y_sb = py.tile([128, 2, KT, BPC, T], BF, tag="y")
            nc.sync.dma_start(y_sb[:], ybf.ap())
            wp_sb = pw.tile([128, 8 * A], BF, tag="wp")
            nc.sync.dma_start(wp_sb[:], wp.ap())
            bp_sb = pw.tile([128, 2], F32, tag="bp")
            nc.sync.dma_start(bp_sb[:], bp2.ap())
            u_sb = pw.tile([128, 2], BF, tag="u")
            nc.sync.dma_start(u_sb[:], u2.ap())
            ones = pw.tile([1, 128], F32, tag="ones")
            nc.vector.memset(ones[:], 1.0)
            msk_sb = [pw.tile([1, T], F32, tag=f"msk{s}", name=f"msk{s}") for s in range(BPC)]
            for s in range(BPC):
                nc.sync.dma_start(msk_sb[s][:], smask.ap()[s:s + 1, :])

            # scores for each sample, on partition 0
            sc = [psc.tile([1, T], F32, tag=f"sc{s}", name=f"sc{s}") for s in range(BPC)]
            for s in range(BPC):
                for ch in range(NC2):
                    pss = pps.tile([1, PC2], F32, tag="pss")
                    for m in range(2):
                        psp = ppp.tile([128, PC2], F32, tag="psp")
                        for dk in range(8):
                            d, k = dk // KT, dk % KT
                            nc.tensor.matmul(
                                psp[:],
                                wp_sb[:, dk * A + m * 128:dk * A + (m + 1) * 128],
                                y_sb[:, d, k, s, ch * PC2:(ch + 1) * PC2],
                                start=(dk == 0), stop=(dk == 7))
                        prj = pt.tile([128, PC2], BF, tag="prj")
                        nc.scalar.activation(prj[:], psp[:], AF.Tanh,
                                             bias=bp_sb[:, m:m + 1])
                        nc.tensor.matmul(pss[:], u_sb[:, m:m + 1], prj[:],
                                         start=(m == 0), stop=(m == 1))
                    nc.vector.tensor_add(sc[s][:, ch * PC2:(ch + 1) * PC2],
                                         pss[:], msk_sb[s][:, ch * PC2:(ch + 1) * PC2])
            if stage < 2:
                for s in range(BPC):
                    nc.sync.dma_start(aout.ap()[s:s + 1, :], sc[s][:])
                wz = pt.tile([128, 8], F32, tag="wz")
                nc.vector.memset(wz[:], 0.0)
                for s in range(BPC):
                    nc.sync.dma_start(wout.ap()[s, :, :], wz[:])
            # softmax per sample on partition 0 + weighted sum
            for s in range(BPC if stage >= 2 else 0):
                mx = pt.tile([1, 1], F32, tag="mx")
                nc.vector.reduce_max(mx[:], sc[s][:], axis=mybir.AxisListType.X)
                nmx = pt.tile([1, 1], F32, tag="nmx")
                nc.vector.tensor_scalar_mul(nmx[:], mx[:], -1.0)
                es = pt.tile([1, T], F32, tag="es")
                nc.scalar.activation(es[:], sc[s][:], AF.Exp, bias=nmx[:])
                sm = pt.tile([1, 1], F32, tag="sm")
                nc.vector.reduce_sum(sm[:], es[:], axis=mybir.AxisListType.X)
                rs = pt.tile([1, 1], F32, tag="rs")
                nc.vector.reciprocal(rs[:], sm[:])
                at = pt.tile([1, T], F32, tag="at")
                nc.vector.tensor_scalar_mul(at[:], es[:], rs[:])
                nc.sync.dma_start(aout.ap()[s:s + 1, :], at[:])
                wacc = pt.tile([128, 8], F32, tag="wacc")
                if stage < 3:
                    nc.vector.memset(wacc[:], 0.0)
                    nc.sync.dma_start(wout.ap()[s, :, :], wacc[:])
                    continue
                for ch in range(NC2):
                    bc = ppb.tile([128, PC2], F32, tag="bc")
                    nc.tensor.matmul(bc[:], ones[:], at[:, ch * PC2:(ch + 1) * PC2],
                                     start=True, stop=True)
                    for dk in range(8):
                        d, k = dk // KT, dk % KT
                        prod = pt.tile([128, PC2], F32, tag="prod")
                        nc.vector.tensor_mul(
                            prod[:],
                            y_sb[:, d, k, s, ch * PC2:(ch + 1) * PC2],
                            bc[:])
                        w1 = pt.tile([128, 1], F32, tag="w1")
                        nc.vector.reduce_sum(w1[:], prod[:],
                                             axis=mybir.AxisListType.X)
                        if ch == 0:
                            nc.vector.tensor_copy(wacc[:, dk:dk + 1], w1[:])
                        else:
                            nc.vector.tensor_add(wacc[:, dk:dk + 1],
                                                 wacc[:, dk:dk + 1], w1[:])
                nc.sync.dma_start(wout.ap()[s, :, :], wacc[:])
    nc.compile()
    return nc


# ------------------------------------------------------------- host prep
def _wT_tiles(w, nkt):
    # [G, K] weight -> [128, nkt * G] bf16, tile (k, m) at col k*G + m*128
    G = w.shape[0]
    return np.ascontiguousarray(
        w.T.reshape(nkt, 128, G).transpose(1, 0, 2).reshape(128, nkt * G)
    ).astype(BF16)


def kernel(x, sequence_lens, Wih_f, Whh_f, bih_f, bhh_f,
           Wih_b, Whh_b, bih_b, bhh_b, Wp, bp, u):
    global LAST_EXEC_NS
    trace = os.environ.get("BASS_KERNEL_TRACE") == "1"
    x = np.asarray(x, np.float32)
    lens = np.asarray(sequence_lens, np.int32)

    if "a" not in _cache:
        _cache["a"] = _build_a()
    if "b" not in _cache:
        _cache["b"] = _build_b()

    # per-direction host-side weight prep
    wih_n = [_wT_tiles(np.asarray(w, np.float32), KT)
             for w in (Wih_f, Wih_b)]
    whh_n = [_wT_tiles(np.asarray(w, np.float32), KT)
             for w in (Whh_f, Whh_b)]
    bprj_n = []
    bhn_n = []
    for bih, bhh in ((bih_f, bhh_f), (bih_b, bhh_b)):
        bih = np.asarray(bih, np.float32)
        bhh = np.asarray(bhh, np.float32)
        btot = bih.copy()
        btot[:2 * H] += bhh[:2 * H]
        bprj_n.append(np.ascontiguousarray(btot.reshape(MT, 128).T))
        bhn_n.append(np.ascontiguousarray(
            np.broadcast_to(bhh[2 * H:].reshape(KT, 128).T[:, :, None],
                            (128, KT, BPC))))

    tt = np.arange(T)
    rev_idx = np.clip(lens[:, None] - 1 - tt[None, :], 0, T - 1)  # [B,T]
    x_rev = np.take_along_axis(x, rev_idx[:, :, None], axis=1)

    def xt_prep(xc):
        # [BPC,T,I] -> [128, KT, BPC*T] bf16
        return np.ascontiguousarray(
            xc.reshape(BPC * T, I).T.reshape(KT, 128, BPC * T)
            .transpose(1, 0, 2)).astype(BF16)

    in_maps_a = []
    for c in range(NCORES):
        sl = slice(c * BPC, (c + 1) * BPC)
        in_maps_a.append({
            "xT": xt_prep(x[sl]), "xrT": xt_prep(x_rev[sl]),
            "wih0": wih_n[0], "wih1": wih_n[1],
            "whh0": whh_n[0], "whh1": whh_n[1],
            "bprj0": bprj_n[0], "bprj1": bprj_n[1],
            "bhn0": bhn_n[0], "bhn1": bhn_n[1],
        })
    res_a = run_bass_kernel_spmd(_cache["a"], in_maps_a,
                                 core_ids=list(range(NCORES)), trace=trace)
    t_a = res_a.exec_time_ns

    # host: un-reverse backward outputs per sample, build launch-B inputs
    wp_n = _wT_tiles(np.asarray(Wp, np.float32), 2 * H // 128)
    bp_n = np.ascontiguousarray(np.asarray(bp, np.float32).reshape(2, 128).T)
    u_n = np.ascontiguousarray(
        np.asarray(u, np.float32)[:, 0].reshape(2, 128).T).astype(BF16)

    in_maps_b = []
    y32_cores = []
    for c in range(NCORES):
        y32 = res_a.results[c]["y32"].copy()   # [128,2,KT,BPC,T] f32
        ybf = res_a.results[c]["ybf"].copy()
        lc = lens[c * BPC:(c + 1) * BPC]
        yb32n = np.zeros_like(y32[:, 1])
        for s in range(BPC):
            l = int(lc[s])
            # forward tail zero + backward un-reverse
            y32[:, 0, :, s, l:] = 0.0
            ybf[:, 0, :, s, l:] = 0.0
            yb32n[:, :, s, :l] = y32[:, 1, :, s, l - 1::-1]
            ybf[:, 1, :, s, :] = 0.0
            ybf[:, 1, :, s, :l] = res_a.results[c]["ybf"][:, 1, :, s, l - 1::-1]
        y32[:, 1] = yb32n
        y32_cores.append(y32)
        smask = np.where(tt[None, :] < lc[:, None], 0.0, -1e30).astype(np.float32)
        in_maps_b.append({"ybf": ybf, "wp": wp_n, "bp2": bp_n, "u2": u_n,
                          "smask": smask})
    res_b = run_bass_kernel_spmd(_cache["b"], in_maps_b,
                                 core_ids=list(range(NCORES)), trace=trace)
    t_b = res_b.exec_time_ns
    LAST_EXEC_NS = (t_a + t_b) if (t_a is not None and t_b is not None) else None

    # host assembly
    weighted = np.zeros((B, 2 * H), np.float32)
    attn = np.zeros((B, T), np.float32)
    out = np.zeros((B, T, 2 * H), np.float32)
    for c in range(NCORES):
        sl = slice(c * BPC, (c + 1) * BPC)
        weighted[sl] = res_b.results[c]["wout"].transpose(0, 2, 1).reshape(BPC, 2 * H)
        attn[sl] = res_b.results[c]["aout"]
        # y32 [128,2,KT,BPC,T] -> out[s, t, d*512 + k*128 + p]
        out[sl] = y32_cores[c].transpose(3, 4, 1, 2, 0).reshape(BPC, T, 2 * H)
    for b_i in range(B):
        out[b_i, lens[b_i]:, :] = 0.0
        attn[b_i, lens[b_i]:] = 0.0
    return weighted, attn, out


# revision 14
# speedup vs baseline: 1.2076x; 1.2076x over previous
"""AttentionRNN (BiGRU + tanh attention pooling) on 8 Trainium2 NeuronCores.

Sharding (v2, direction-split): cores 0-3 run the FORWARD GRU for 8 samples
each; cores 4-7 run the BACKWARD GRU for the same sample groups on
host-flipped input (np.flip over the full time axis - static, not ragged).
A per-step mask multiply (h <- mask * GRU(h, xg)) reproduces exact
pack semantics: on backward cores the state stays zero until the scan
enters each sample's valid region, so every core runs the identical
upward-scanning SPMD program and only the input data differs.

Launch A: input projection (bf16 matmuls, fp32 accum) + the masked GRU
scan (one direction, batch 8, gate tiles stationary on the PE).
Launch B: attention (proj tanh / scores / masked softmax / weighted sum),
data-parallel 4 samples per core; the host routes y between launches
(flip of backward outputs + concat) and assembles the final outputs.
"""

import os
import sys
import types

import numpy as np
import ml_dtypes

BF16 = ml_dtypes.bfloat16

B, T, I, H, A = 32, 1024, 512, 512, 256
NCORES = 8
S = 8                      # samples per core in launch A (direction-split)
BPC = 4                    # samples per core in launch B
KT = I // 128              # 4 k-tiles for the 512-dim contractions
MT = 3 * H // 128          # 12 gate m-tiles
CHUNK = 64                 # scan steps per xg/y SBUF chunk


def _install_ntff_shim():
    try:
        from antenv import axon_hooks  # noqa: F401
        return
    except ImportError:
        pass
    try:
        import antenv
        from trn_agent_boot.trn_boot import _ntff_profile_via_ctypes
        hook = _ntff_profile_via_ctypes('/opt/axon/libaxon_pjrt.so')
        m = types.ModuleType('antenv.axon_hooks')
        m.get_axon_ntff_profile_hook = lambda: hook
        m.set_axon_ntff_profile_hook = lambda h: None
        sys.modules['antenv.axon_hooks'] = m
        antenv.axon_hooks = m
    except Exception:
        pass


_install_ntff_shim()

import concourse.bass as bass  # noqa: E402
import concourse.mybir as mybir  # noqa: E402
import concourse.tile as tile  # noqa: E402
from concourse import bacc  # noqa: E402
from concourse.bass_utils import run_bass_kernel_spmd as _run_spmd  # noqa: E402


def run_bass_kernel_spmd(nc, in_maps, core_ids, trace=False):
    import time as _time
    last = None
    for attempt in range(3):
        try:
            return _run_spmd(nc, in_maps, core_ids=core_ids, trace=trace)
        except Exception as e:  # wedged-device flakes on shared cores
            last = e
            _time.sleep(3.0)
    raise last


F32 = mybir.dt.float32
BF = mybir.dt.bfloat16
AF = mybir.ActivationFunctionType

_cache = {}
LAST_EXEC_NS = None


# ---------------------------------------------------------------- launch A
def _build_a():
    nc = bacc.Bacc("TRN2", target_bir_lowering=False, debug=False,
                   num_devices=NCORES)
    xT = nc.dram_tensor("xT", [128, KT, S * T], BF, kind="ExternalInput")
    wih = nc.dram_tensor("wih", [128, KT * 3 * H], BF, kind="ExternalInput")
    whh = nc.dram_tensor("whh", [128, KT * 3 * H], BF, kind="ExternalInput")
    bprj = nc.dram_tensor("bprj", [128, MT], F32, kind="ExternalInput")
    bhn = nc.dram_tensor("bhn", [128, KT, S], F32, kind="ExternalInput")
    msk = nc.dram_tensor("msk", [128, KT, S, T], BF, kind="ExternalInput")
    y32 = nc.dram_tensor("y32", [128, KT, S, T], F32, kind="ExternalOutput")
    ybf = nc.dram_tensor("ybf", [128, KT, S, T], BF, kind="ExternalOutput")
    xg = nc.dram_tensor("xg", [S, MT, 128, T], F32, kind="Internal")

    with tile.TileContext(nc) as tc:
        # ---- phase 1: input projections xg = Wih @ x^T + (bih [+ bhh]) ----
        with tc.tile_pool(name="pw", bufs=1) as pw, \
             tc.tile_pool(name="px", bufs=1) as px, \
             tc.tile_pool(name="pp", bufs=4, space="PSUM") as pp, \
             tc.tile_pool(name="po", bufs=4) as po:
            x_sb = px.tile([128, KT, S * T], BF, tag="x")
            nc.sync.dma_start(x_sb[:], xT.ap())
            w_sb = pw.tile([128, KT * 3 * H], BF, tag="w")
            nc.sync.dma_start(w_sb[:], wih.ap())
            b_sb = pw.tile([128, MT], F32, tag="b")
            nc.sync.dma_start(b_sb[:], bprj.ap())
            PC = min(512, T)
            nch = S * T // PC
            for m in range(MT):
                for ch in range(nch):
                    ps = pp.tile([128, PC], F32, tag="ps", name=f"ps_{m}_{ch}")
                    for k in range(KT):
                        nc.tensor.matmul(
                            ps[:],
                            w_sb[:, k * 3 * H + m * 128:
                                 k * 3 * H + (m + 1) * 128],
                            x_sb[:, k, ch * PC:(ch + 1) * PC],
                            start=(k == 0), stop=(k == KT - 1))
                    ot = po.tile([128, PC], F32, tag="ot", name=f"ot_{m}_{ch}")
                    nc.vector.tensor_scalar_add(ot[:], ps[:], b_sb[:, m:m + 1])
                    s = (ch * PC) // T
                    tl = (ch * PC) % T
                    nc.sync.dma_start(xg.ap()[s, m, :, tl:tl + PC], ot[:])

        # ---- phase 2: the masked GRU scan (one direction, batch S) ----
        tc.strict_bb_all_engine_barrier()
        with tc.tile_pool(name="sw", bufs=1) as sw, \
             tc.tile_pool(name="sxg", bufs=2) as sxg, \
             tc.tile_pool(name="smk", bufs=2) as smk, \
             tc.tile_pool(name="sy32", bufs=2) as sy32, \
             tc.tile_pool(name="sybf", bufs=2) as sybf, \
             tc.tile_pool(name="sps", bufs=2, space="PSUM") as sps, \
             tc.tile_pool(name="stmp", bufs=3) as stmp:
            whh_sb = sw.tile([128, KT * 3 * H], BF, tag="whh")
            nc.sync.dma_start(whh_sb[:], whh.ap())
            bhn_sb = sw.tile([128, KT, S], F32, tag="bhn")
            nc.sync.dma_start(bhn_sb[:], bhn.ap())
            z32 = sw.tile([128, KT, S], F32, tag="z32")
            nc.vector.memset(z32[:], 0.0)
            zbf = sw.tile([128, KT, S], BF, tag="zbf")
            nc.vector.memset(zbf[:], 0.0)

            prev32, prevbf = z32, zbf
            nchunks = T // CHUNK
            for ci in range(nchunks):
                t0 = ci * CHUNK
                xgc = sxg.tile([128, MT, S, CHUNK], F32, tag="xg",
                               name=f"xgc_{ci}")
                for s in range(S):
                    for m in range(MT):
                        nc.sync.dma_start(xgc[:, m, s, :],
                                          xg.ap()[s, m, :, t0:t0 + CHUNK])
                mkc = smk.tile([128, KT, S, CHUNK], BF, tag="mk",
                               name=f"mkc_{ci}")
                nc.sync.dma_start(mkc[:], msk.ap()[:, :, :, t0:t0 + CHUNK])
                yb32 = sy32.tile([128, KT, S, CHUNK], F32, tag="y32",
                                 name=f"yb32_{ci}")
                ybbf = sybf.tile([128, KT, S, CHUNK], BF, tag="ybf",
                                 name=f"ybbf_{ci}")
                for tl in range(CHUNK):
                    first = (ci == 0 and tl == 0)
                    if tl == 0:
                        hbf_t, h32_t = prevbf, prev32
                    else:
                        hbf_t, h32_t = ybbf, yb32

                    def hbf_rhs(k):
                        if first:
                            return hbf_t[:, k, :]
                        if tl == 0:
                            return hbf_t[:, k, :, CHUNK - 1]
                        return hbf_t[:, k, :, tl - 1]

                    if first:
                        h32_ap = h32_t[:, :, :]
                    elif tl == 0:
                        h32_ap = h32_t[:, :, :, CHUNK - 1]
                    else:
                        h32_ap = h32_t[:, :, :, tl - 1]
                    ps = sps.tile([128, MT, S], F32, tag="ps",
                                  name=f"sps_{ci}_{tl}")
                    for m in range(MT):
                        for k in range(KT):
                            nc.tensor.matmul(
                                ps[:, m, :],
                                whh_sb[:, k * 3 * H + m * 128:
                                       k * 3 * H + (m + 1) * 128],
                                hbf_rhs(k),
                                start=(k == 0), stop=(k == KT - 1))
                    xga = xgc[:, :, :, tl]
                    # n-gate hidden bias (sits inside the r* product)
                    nc.vector.tensor_add(ps[:, 8:12, :], ps[:, 8:12, :],
                                         bhn_sb[:])
                    srz = stmp.tile([128, 8, S], F32, tag="srz",
                                    name=f"srz_{ci}_{tl}")
                    nc.vector.tensor_add(srz[:], ps[:, 0:8, :], xga[:, 0:8, :])
                    rz = stmp.tile([128, 8, S], F32, tag="rz",
                                   name=f"rz_{ci}_{tl}")
                    nc.scalar.activation(rz[:], srz[:], AF.Sigmoid)
                    tn = stmp.tile([128, KT, S], F32, tag="tn",
                                   name=f"tn_{ci}_{tl}")
                    nc.vector.tensor_mul(tn[:], rz[:, 0:4, :], ps[:, 8:12, :])
                    nc.vector.tensor_add(tn[:], tn[:], xga[:, 8:12, :])
                    n_t = stmp.tile([128, KT, S], F32, tag="n",
                                    name=f"n_{ci}_{tl}")
                    nc.scalar.activation(n_t[:], tn[:], AF.Tanh)
                    dd = stmp.tile([128, KT, S], F32, tag="dd",
                                   name=f"dd_{ci}_{tl}")
                    nc.vector.tensor_sub(dd[:], h32_ap, n_t[:])
                    nc.vector.tensor_mul(dd[:], rz[:, 4:8, :], dd[:])
                    nc.vector.tensor_add(dd[:], n_t[:], dd[:])
                    # pack-semantics mask: freeze-at-zero outside valid region
                    nc.vector.tensor_mul(yb32[:, :, :, tl], dd[:],
                                         mkc[:, :, :, tl])
                    nc.scalar.copy(ybbf[:, :, :, tl], yb32[:, :, :, tl])
                nc.sync.dma_start(y32.ap()[:, :, :, t0:t0 + CHUNK], yb32[:])
                nc.sync.dma_start(ybf.ap()[:, :, :, t0:t0 + CHUNK], ybbf[:])
                prev32, prevbf = yb32, ybbf
    nc.compile()
    return nc


# ---------------------------------------------------------------- launch B
def _build_b(stage=3):
    nc = bacc.Bacc("TRN2", target_bir_lowering=False, debug=False,
                   num_devices=NCORES)
    ybf = nc.dram_tensor("ybf", [128, 2, KT, BPC, T], BF, kind="ExternalInput")
    wp = nc.dram_tensor("wp", [128, 8 * A], BF, kind="ExternalInput")
    bp2 = nc.dram_tensor("bp2", [128, 2], F32, kind="ExternalInput")
    u2 = nc.dram_tensor("u2", [128, 2], BF, kind="ExternalInput")
    smask = nc.dram_tensor("smask", [BPC, T], F32, kind="ExternalInput")
    wout = nc.dram_tensor("wout", [BPC, 128, 8], F32, kind="ExternalOutput")
    aout = nc.dram_tensor("aout", [BPC, T], F32, kind="ExternalOutput")

    PC2 = min(512, T)
    NC2 = T // PC2
    with tile.TileContext(nc) as tc:
        with tc.tile_pool(name="py", bufs=1) as py, \
             tc.tile_pool(name="pw", bufs=1) as pw, \
             tc.tile_pool(name="ppp", bufs=2, space="PSUM") as ppp, \
             tc.tile_pool(name="pps", bufs=2, space="PSUM") as pps, \
             tc.tile_pool(name="ppb", bufs=2, space="PSUM") as ppb, \
             tc.tile_pool(name="pt", bufs=4) as pt, \
             tc.tile_pool(name="psc", bufs=1) as psc:
            y_sb = py.tile([128, 2, KT, BPC, T], BF, tag="y")
            nc.sync.dma_start(y_sb[:], ybf.ap())
            wp_sb = pw.tile([128, 8 * A], BF, tag="wp")
            nc.sync.dma_start(wp_sb[:], wp.ap())
            bp_sb = pw.tile([128, 2], F32, tag="bp")
            nc.sync.dma_start(bp_sb[:], bp2.ap())
            u_sb = pw.tile([128, 2], BF, tag="u")
            nc.sync.dma_start(u_sb[:], u2.ap())
            ones = pw.tile([1, 128], F32, tag="ones")
            nc.vector.memset(ones[:], 1.0)
            msk_sb = [pw.tile([1, T], F32, tag=f"msk{s}", name=f"msk{s}")
                      for s in range(BPC)]
            for s in range(BPC):
                nc.sync.dma_start(msk_sb[s][:], smask.ap()[s:s + 1, :])

            # scores for each sample, on partition 0
            sc = [psc.tile([1, T], F32, tag=f"sc{s}", name=f"sc{s}")
                  for s in range(BPC)]
            for s in range(BPC):
                for ch in range(NC2):
                    pss = pps.tile([1, PC2], F32, tag="pss")
                    for m in range(2):
                        psp = ppp.tile([128, PC2], F32, tag="psp")
                        for dk in range(8):
                            d, k = dk // KT, dk % KT
                            nc.tensor.matmul(
                                psp[:],
                                wp_sb[:, dk * A + m * 128:dk * A + (m + 1) * 128],
                                y_sb[:, d, k, s, ch * PC2:(ch + 1) * PC2],
                                start=(dk == 0), stop=(dk == 7))
                        prj = pt.tile([128, PC2], BF, tag="prj")
                        nc.scalar.activation(prj[:], psp[:], AF.Tanh,
                                             bias=bp_sb[:, m:m + 1])
                        nc.tensor.matmul(pss[:], u_sb[:, m:m + 1], prj[:],
                                         start=(m == 0), stop=(m == 1))
                    nc.vector.tensor_add(sc[s][:, ch * PC2:(ch + 1) * PC2],
                                         pss[:],
                                         msk_sb[s][:, ch * PC2:(ch + 1) * PC2])
            # softmax per sample on partition 0 + weighted sum
            for s in range(BPC):
                mx = pt.tile([1, 1], F32, tag="mx")
                nc.vector.reduce_max(mx[:], sc[s][:], axis=mybir.AxisListType.X)
                nmx = pt.tile([1, 1], F32, tag="nmx")
                nc.vector.tensor_scalar_mul(nmx[:], mx[:], -1.0)
                es = pt.tile([1, T], F32, tag="es")
                nc.scalar.activation(es[:], sc[s][:], AF.Exp, bias=nmx[:])
                sm = pt.tile([1, 1], F32, tag="sm")
                nc.vector.reduce_sum(sm[:], es[:], axis=mybir.AxisListType.X)
                rs = pt.tile([1, 1], F32, tag="rs")
                nc.vector.reciprocal(rs[:], sm[:])
                at = pt.tile([1, T], F32, tag="at")
                nc.vector.tensor_scalar_mul(at[:], es[:], rs[:])
                nc.sync.dma_start(aout.ap()[s:s + 1, :], at[:])
                wacc = pt.tile([128, 8], F32, tag="wacc")
                for ch in range(NC2):
                    bc = ppb.tile([128, PC2], F32, tag="bc")
                    nc.tensor.matmul(bc[:], ones[:],
                                     at[:, ch * PC2:(ch + 1) * PC2],
                                     start=True, stop=True)
                    for dk in range(8):
                        d, k = dk // KT, dk % KT
                        prod = pt.tile([128, PC2], F32, tag="prod")
                        nc.vector.tensor_mul(
                            prod[:],
                            y_sb[:, d, k, s, ch * PC2:(ch + 1) * PC2],
                            bc[:])
                        w1 = pt.tile([128, 1], F32, tag="w1")
                        nc.vector.reduce_sum(w1[:], prod[:],
                                             axis=mybir.AxisListType.X)
                        if ch == 0:
                            nc.vector.tensor_copy(wacc[:, dk:dk + 1], w1[:])
                        else:
                            nc.vector.tensor_add(wacc[:, dk:dk + 1],
                                                 wacc[:, dk:dk + 1], w1[:])
                nc.sync.dma_start(wout.ap()[s, :, :], wacc[:])
    nc.compile()
    return nc


# ------------------------------------------------------------- host prep
def _wT_tiles(w, nkt):
    # [G, K] weight -> [128, nkt * G] bf16, tile (k, m) at col k*G + m*128
    G = w.shape[0]
    return np.ascontiguousarray(
        w.T.reshape(nkt, 128, G).transpose(1, 0, 2).reshape(128, nkt * G)
    ).astype(BF16)


def kernel(x, sequence_lens, Wih_f, Whh_f, bih_f, bhh_f,
           Wih_b, Whh_b, bih_b, bhh_b, Wp, bp, u):
    global LAST_EXEC_NS
    trace = os.environ.get("BASS_KERNEL_TRACE") == "1"
    x = np.asarray(x, np.float32)
    lens = np.asarray(sequence_lens, np.int32)

    if "a" not in _cache:
        _cache["a"] = _build_a()
    if "b" not in _cache:
        _cache["b"] = _build_b()

    wih_n = [_wT_tiles(np.asarray(w, np.float32), KT)
             for w in (Wih_f, Wih_b)]
    whh_n = [_wT_tiles(np.asarray(w, np.float32), KT)
             for w in (Whh_f, Whh_b)]
    bprj_n = []
    bhn_half = []
    for bih, bhh in ((bih_f, bhh_f), (bih_b, bhh_b)):
        bih = np.asarray(bih, np.float32)
        bhh = np.asarray(bhh, np.float32)
        btot = bih.copy()
        btot[:2 * H] += bhh[:2 * H]
        bprj_n.append(np.ascontiguousarray(btot.reshape(MT, 128).T))
        bhn_half.append(np.ascontiguousarray(np.broadcast_to(
            bhh[2 * H:].reshape(KT, 128).T[:, :, None], (128, KT, S))))

    tt = np.arange(T)
    x_flip = np.ascontiguousarray(x[:, ::-1, :])

    def xt_prep(xc):
        # [S,T,I] -> [128, KT, S*T] bf16
        return np.ascontiguousarray(
            xc.reshape(S * T, I).T.reshape(KT, 128, S * T)
            .transpose(1, 0, 2)).astype(BF16)

    in_maps_a = []
    for c in range(NCORES):
        d = c // 4          # 0: forward cores, 1: backward cores
        g = c % 4           # sample group
        sl = slice(g * S, (g + 1) * S)
        lc = lens[sl]
        if d == 0:
            xc = x[sl]
            mval = tt[None, :] < lc[:, None]            # [S,T]
        else:
            xc = x_flip[sl]
            mval = (T - 1 - tt)[None, :] < lc[:, None]
        mbc = np.broadcast_to(mval[None, None, :, :].astype(BF16),
                              (128, KT, S, T))
        in_maps_a.append({
            "xT": xt_prep(xc),
            "wih": wih_n[d], "whh": whh_n[d],
            "bprj": bprj_n[d], "bhn": bhn_half[d],
            "msk": np.ascontiguousarray(mbc),
        })
    res_a = run_bass_kernel_spmd(_cache["a"], in_maps_a,
                                 core_ids=list(range(NCORES)), trace=trace)
    t_a = res_a.exec_time_ns

    # host: route y between launches (flip backward cores' outputs)
    wp_n = _wT_tiles(np.asarray(Wp, np.float32), 2 * H // 128)
    bp_n = np.ascontiguousarray(np.asarray(bp, np.float32).reshape(2, 128).T)
    u_n = np.ascontiguousarray(
        np.asarray(u, np.float32)[:, 0].reshape(2, 128).T).astype(BF16)

    yf32 = [res_a.results[c]["y32"] for c in range(4)]          # [128,KT,S,T]
    yb32 = [res_a.results[c]["y32"][:, :, :, ::-1] for c in range(4, 8)]
    yfbf = [res_a.results[c]["ybf"] for c in range(4)]
    ybbf = [res_a.results[c]["ybf"][:, :, :, ::-1] for c in range(4, 8)]

    in_maps_b = []
    for cb in range(NCORES):
        g, half = cb // 2, cb % 2
        ssl = slice(half * BPC, (half + 1) * BPC)   # within the S=8 group
        yb = np.empty((128, 2, KT, BPC, T), BF16)
        yb[:, 0] = yfbf[g][:, :, ssl, :]
        yb[:, 1] = ybbf[g][:, :, ssl, :]
        lc = lens[g * S + half * BPC: g * S + (half + 1) * BPC]
        smask = np.where(tt[None, :] < lc[:, None], 0.0, -1e30).astype(np.float32)
        in_maps_b.append({"ybf": yb, "wp": wp_n, "bp2": bp_n, "u2": u_n,
                          "smask": smask})
    res_b = run_bass_kernel_spmd(_cache["b"], in_maps_b,
                                 core_ids=list(range(NCORES)), trace=trace)
    t_b = res_b.exec_time_ns
    LAST_EXEC_NS = (t_a + t_b) if (t_a is not None and t_b is not None) else None

    # host assembly
    weighted = np.zeros((B, 2 * H), np.float32)
    attn = np.zeros((B, T), np.float32)
    out = np.zeros((B, T, 2 * H), np.float32)
    for cb in range(NCORES):
        g, half = cb // 2, cb % 2
        sl = slice(g * S + half * BPC, g * S + (half + 1) * BPC)
        weighted[sl] = res_b.results[cb]["wout"].transpose(0, 2, 1).reshape(
            BPC, 2 * H)
        attn[sl] = res_b.results[cb]["aout"]
    for g in range(4):
        sl = slice(g * S, (g + 1) * S)
        # y [128,KT,S,T] -> [S,T,KT,128] -> features k*128+p
        out[sl, :, :H] = yf32[g].transpose(2, 3, 1, 0).reshape(S, T, H)
        out[sl, :, H:] = yb32[g].transpose(2, 3, 1, 0).reshape(S, T, H)
    return weighted, attn, out
